# revision 1
# baseline (speedup 1.0000x reference)
"""Self-contained Trainium2 Bass kernel for nn_Attention_23776938951493.

Computation (see reference): LayerNorm -> q/k/v projections -> flat-reshape
attention (head h attends over tokens [128h, 128(h+1)) reshaped to [2048, 64])
-> out projection.  The flat reshape makes every (batch, head) pair an
independent 128-token block: 32 blocks total, 4 per NeuronCore, no
collectives needed.

v4: generator-interleaved emission — the next block's LayerNorm /
projections are woven between the current block's scores/exp/AV steps so
the PE always has dense back-to-back matmul work (keeps the HAM clock
warm through the ACT-bound softmax phase).  Scores matmuls alternate
low/high PE row-groups so consecutive instructions execute concurrently
(measured dstart 3-7 ns).  fp16 operand path, softmax denominator via an
appended ones-column in V.
"""

import os
import sys

sys.path.insert(0, "/opt/trn_rl_repo")
os.environ.setdefault("JAX_PLATFORMS", "axon")

import numpy as np
from contextlib import ExitStack

B, N, D = 2, 2048, 1024
H, DH = 16, 64
NCORES = 8
BLK = 128      # tokens per block
BPC = 4        # blocks per core
LN_EPS = 1e-5

_compiled = {}


def _build(has_bias: bool):
    import concourse.bass as bass
    import concourse.tile as tile
    from concourse import bacc, mybir
    from concourse.masks import make_identity

    f32 = mybir.dt.float32
    f32r = mybir.dt.float32r
    fp16 = mybir.dt.float16
    FT = mybir.ActivationFunctionType
    sub = mybir.AluOpType.subtract
    mult = mybir.AluOpType.mult

    nc = bacc.Bacc("TRN2", target_bir_lowering=False, debug=False,
                   num_devices=NCORES)
    xs = nc.dram_tensor("xs", [BPC * BLK, D], f32, kind="ExternalInput").ap()
    wdr = {}
    for nm in ("q", "k", "v", "o"):
        wdr[nm] = nc.dram_tensor(f"w{nm}", [D, D], fp16,
                                 kind="ExternalInput").ap()
    if has_bias:
        bqk_dr = nc.dram_tensor("bqk", [128, 16], f32,
                                kind="ExternalInput").ap()
        bv_dr = nc.dram_tensor("bv", [1, D], f32, kind="ExternalInput").ap()
    out_dr = nc.dram_tensor("out", [BPC * BLK, D], f32,
                            kind="ExternalOutput").ap()

    with tile.TileContext(nc) as tc:
        with ExitStack() as ctx:
            P = lambda name, bufs, **kw: ctx.enter_context(
                tc.tile_pool(name=name, bufs=bufs, **kw))
            consts = P("consts", 1)
            wpool = P("w", 1)
            xpool = P("x", 2)
            xnpool = P("xn", 2)
            xntp = P("xnt", 2)
            qkp = P("qk", 2)
            yvp = P("yv", 2)
            ep = P("e", 4)
            avsp = P("avs", 2)
            scrp = P("scr", 2)
            avp = P("av", 2)
            avtp = P("avt", 2)
            outp = P("out", 2)
            psf = P("psf", 3, space="PSUM")
            psav = P("psav", 1, space="PSUM")

            # identities for PE transposes (producer dtype must match the
            # consuming matmul's rounded dtype)
            idscratch = outp.tile([128, 1024], f32, tag="out")
            make_identity(nc, idscratch[:, 0:128])
            ident = consts.tile([128, 128], f32r, tag="ident")
            nc.vector.tensor_copy(out=ident[:], in_=idscratch[:, 0:128])
            ident16 = consts.tile([128, 128], fp16, tag="ident16")
            nc.vector.tensor_copy(out=ident16[:], in_=idscratch[:, 0:128])

            # resident fp16 weights, chunk-major; q first so the first
            # projection can start as early as possible.  x/out DMAs keep
            # the sync queue to themselves.
            W = {}
            engs = [nc.gpsimd, nc.scalar]
            for wi, nm in enumerate(("q", "k", "v", "o")):
                W[nm] = wpool.tile([128, 8, 1024], fp16, tag=f"w{nm}",
                                   name=f"w{nm}")
                for j in range(8):
                    engs[j % 2].dma_start(
                        out=W[nm][:, j, :], in_=wdr[nm][128 * j:128 * (j + 1), :])
            if has_bias:
                bqk = consts.tile([128, 16], f32, tag="bqk")
                nc.sync.dma_start(out=bqk[:], in_=bqk_dr[:])
                bvb = consts.tile([128, D], f32, tag="bvb")
                nc.gpsimd.dma_start(out=bvb[:], in_=bv_dr.broadcast_to((128, D)))

            blocks = {}

            def phase1(i):
                """LN + projections for block i; yields between chunks."""
                xi = xpool.tile([128, D], f32, tag="x", name="xi")
                nc.sync.dma_start(out=xi[:], in_=xs[BLK * i:BLK * (i + 1), :])
                scr = scrp.tile([128, 32], f32, tag="scr", name="scr")
                nc.vector.memset(scr[:, 0:1], LN_EPS)
                sview = scr[:, 1:13].rearrange("p (s d) -> p s d", d=6)
                for s in range(2):
                    nc.vector.bn_stats(out=sview[:, s, :],
                                       in_=xi[:, 512 * s:512 * (s + 1)])
                nc.vector.bn_aggr(out=scr[:, 13:15], in_=sview)
                # rstd = exp(-0.5*ln(var+eps)): stays on the ln/exp ACT set
                nc.scalar.activation(out=scr[:, 15:16], in_=scr[:, 14:15],
                                     func=FT.Ln, bias=scr[:, 0:1], scale=1.0)
                nc.scalar.activation(out=scr[:, 30:31], in_=scr[:, 15:16],
                                     func=FT.Exp, scale=-0.5)
                xn = xnpool.tile([128, D], fp16, tag="xn", name="xn")
                nc.vector.tensor_scalar(out=xn[:], in0=xi[:],
                                        scalar1=scr[:, 13:14],
                                        scalar2=scr[:, 30:31],
                                        op0=sub, op1=mult)
                yield
                xnt = xntp.tile([128, 8, 128], fp16, tag="xnt", name="xnt")
                qt = qkp.tile([128, 16, 128], fp16, tag="qt", name="qt")
                kt = qkp.tile([128, 16, 128], fp16, tag="kt", name="kt")
                yva = yvp.tile([128, 16, 65], fp16, tag="yva", name="yva")
                blocks[i] = (scr, xnt, qt, kt, yva)
                for j in range(8):
                    pt = psf.tile([128, 2048], fp16, tag="flex", name="pt")
                    nc.tensor.transpose(pt[:, 0:128],
                                        xn[:, 128 * j:128 * (j + 1)],
                                        ident16[:])
                    nc.vector.tensor_copy(out=xnt[:, j, :], in_=pt[:, 0:128])
                    if j % 2:
                        yield
                # q/k projections (Y^T computed directly); head-group slabs
                # duplicated on both partition halves for row-group pairing
                for nm, dst in (("q", qt), ("k", kt)):
                    for obp in range(4):
                        pp = psf.tile([128, 1024], f32, tag="flex", name="pp")
                        for osub in range(2):
                            ob = 2 * obp + osub
                            for j in range(8):
                                nc.tensor.matmul(
                                    pp[:, 128 * osub:128 * (osub + 1)],
                                    W[nm][:, j, 128 * ob:128 * (ob + 1)],
                                    xnt[:, j, :], start=(j == 0), stop=(j == 7))
                            if has_bias:
                                col = (0 if nm == "q" else 8) + ob
                                nc.vector.tensor_scalar_add(
                                    out=pp[:, 128 * osub:128 * (osub + 1)],
                                    in0=pp[:, 128 * osub:128 * (osub + 1)],
                                    scalar1=bqk[:, col:col + 1])
                        ppv = pp[:, 0:256].rearrange("p (s t) -> p s t", s=2)
                        nc.vector.tensor_copy(
                            out=dst[0:64, 4 * obp:4 * obp + 4:2, :],
                            in_=ppv[0:64, :, :])
                        nc.vector.tensor_copy(
                            out=dst[64:128, 4 * obp + 1:4 * obp + 4:2, :],
                            in_=ppv[64:128, :, :])
                        nc.gpsimd.dma_start(
                            out=dst[64:128, 4 * obp:4 * obp + 4:2, :],
                            in_=dst[0:64, 4 * obp:4 * obp + 4:2, :])
                        nc.sync.dma_start(
                            out=dst[0:64, 4 * obp + 1:4 * obp + 4:2, :],
                            in_=dst[64:128, 4 * obp + 1:4 * obp + 4:2, :])
                        yield
                # v projection -> Yv [t, o] plus ones column per group
                pv = psf.tile([128, 1024], f32, tag="flex", name="pv")
                for hh in range(2):
                    for j in range(8):
                        nc.tensor.matmul(
                            pv[:, 512 * hh:512 * (hh + 1)], xnt[:, j, :],
                            W["v"][:, j, 512 * hh:512 * (hh + 1)],
                            start=(j == 0), stop=(j == 7))
                    yield
                if has_bias:
                    nc.vector.tensor_add(out=pv[:], in0=pv[:], in1=bvb[:])
                nc.vector.memset(yva[:, :, 64:65], 1.0)
                for hh in range(2):
                    nc.vector.tensor_copy(
                        out=yva[:, 8 * hh:8 * (hh + 1), 0:64],
                        in_=pv[:, 512 * hh:512 * (hh + 1)].rearrange(
                            "p (g d) -> p g d", d=64))
                yield

            def phase2(i):
                """scores -> exp -> AV -> normalize -> out proj for block i."""
                scr, xnt, qt, kt, yva = blocks[i]
                av = avp.tile([128, 1024], fp16, tag="av", name="av")
                for hh in range(2):
                    pa = psav.tile([65, 8, 128], f32, tag="psav", name="pa")
                    for gp in range(8):
                        e2 = ep.tile([128, 2, 8, 128], fp16, tag="e",
                                     name="e2")
                        scs = [psf.tile([128, 1024], f32, tag="flex",
                                        name=f"sc{s}") for s in range(2)]
                        # alternate low/high row-groups so consecutive MMs
                        # run concurrently on the PE array halves
                        for q4 in range(2):
                            for side in range(2):
                                g2 = 2 * gp + side
                                base = 64 * side
                                nc.tensor.matmul(
                                    scs[side][:, 512 * q4:512 * (q4 + 1)],
                                    kt[base:base + 64, g2, :],
                                    qt[base:base + 64,
                                       8 * hh + 4 * q4:8 * hh + 4 * (q4 + 1), :],
                                    start=True, stop=True)
                        for side in range(2):
                            g2 = 2 * gp + side
                            nc.scalar.activation(
                                out=e2[:, side, :, :], in_=scs[side][:],
                                func=FT.Exp, scale=0.125)
                            for q4 in range(2):
                                nc.tensor.matmul(
                                    pa[:, 4 * q4:4 * (q4 + 1), :],
                                    yva[:, g2, :],
                                    e2[:, side, 4 * q4:4 * (q4 + 1), :],
                                    start=(gp == 0 and side == 0),
                                    stop=(gp == 7 and side == 1))
                        yield

                    # normalize this half and assemble AV [t, 1024]
                    # (rows 65:128 of avs are never initialized: the
                    # transpose maps them to output cols 65:128, unread)
                    avs = avsp.tile([128, 8, 128], f32r, tag="avs", name="avs")
                    nc.vector.tensor_copy(out=avs[0:65, :, :], in_=pa[:])
                    for c in range(8):
                        tr = psf.tile([128, 1024], f32, tag="flex", name="tr")
                        nc.tensor.transpose(tr[:, 0:128].bitcast(f32r),
                                            avs[:, c, :], ident[:])
                        rc = 16 + 8 * hh + c
                        nc.vector.reciprocal(out=scr[:, rc:rc + 1],
                                             in_=tr[:, 64:65])
                        g = 8 * hh + c
                        nc.vector.tensor_scalar_mul(
                            out=av[:, 64 * g:64 * (g + 1)],
                            in0=tr[:, 0:64], scalar1=scr[:, rc:rc + 1])
                    yield

                # out projection (fp16 operands, fp32 accumulate)
                avt = avtp.tile([128, 8, 128], fp16, tag="avt", name="avt")
                for j in range(8):
                    pt = psf.tile([128, 2048], fp16, tag="flex", name="pt2")
                    nc.tensor.transpose(pt[:, 0:128],
                                        av[:, 128 * j:128 * (j + 1)],
                                        ident16[:])
                    nc.vector.tensor_copy(out=avt[:, j, :], in_=pt[:, 0:128])
                yield
                po = psf.tile([128, 1024], f32, tag="flex", name="po")
                for hh in range(2):
                    for j in range(8):
                        nc.tensor.matmul(
                            po[:, 512 * hh:512 * (hh + 1)], avt[:, j, :],
                            W["o"][:, j, 512 * hh:512 * (hh + 1)],
                            start=(j == 0), stop=(j == 7))
                ob_t = outp.tile([128, 1024], f32, tag="out", name="ob")
                nc.vector.tensor_copy(out=ob_t[:], in_=po[:])
                nc.sync.dma_start(out=out_dr[BLK * i:BLK * (i + 1), :],
                                  in_=ob_t[:])
                yield

            # drive: weave block i+1's phase 1 between block i's phase-2
            # steps so the PE never starves while ACT chews on the exps
            def drain(g):
                if g is not None:
                    for _ in g:
                        pass

            g1 = phase1(0)
            drain(g1)
            for b in range(BPC):
                g2 = phase2(b)
                g1 = phase1(b + 1) if b + 1 < BPC else None
                while True:
                    try:
                        next(g2)
                    except StopIteration:
                        break
                    if g1 is not None:
                        for _ in range(2):
                            try:
                                next(g1)
                            except StopIteration:
                                g1 = None
                                break
                drain(g1)

    nc.compile()
    return nc


def _get(has_bias: bool):
    if has_bias not in _compiled:
        _compiled[has_bias] = _build(has_bias)
    return _compiled[has_bias]


def _in_maps(x, gamma, beta, Wq, Wk, Wv, Wo):
    wq_t = np.ascontiguousarray((Wq * gamma[None, :]).T.astype(np.float16))
    wk_t = np.ascontiguousarray((Wk * gamma[None, :]).T.astype(np.float16))
    wv_t = np.ascontiguousarray((Wv * gamma[None, :]).T.astype(np.float16))
    wo_t = np.ascontiguousarray(Wo.T.astype(np.float16))
    has_bias = bool(np.any(beta))
    maps = []
    for c in range(NCORES):
        blocks = [x[g // 16, 128 * (g % 16):128 * (g % 16 + 1), :]
                  for g in range(BPC * c, BPC * (c + 1))]
        m = {"xs": np.ascontiguousarray(np.concatenate(blocks, axis=0)),
             "wq": wq_t, "wk": wk_t, "wv": wv_t, "wo": wo_t}
        if has_bias:
            bq = beta @ Wq.T
            bk = beta @ Wk.T
            bv = beta @ Wv.T
            m["bqk"] = np.ascontiguousarray(np.concatenate(
                [bq.reshape(8, 128).T, bk.reshape(8, 128).T], axis=1))
            m["bv"] = np.ascontiguousarray(bv.reshape(1, D))
        maps.append(m)
    return maps, has_bias


def kernel(x, gamma, beta, Wq, Wk, Wv, Wo):
    from concourse.bass_utils import run_bass_kernel_spmd

    x = np.ascontiguousarray(np.asarray(x, dtype=np.float32))
    gamma = np.asarray(gamma, dtype=np.float32)
    beta = np.asarray(beta, dtype=np.float32)
    Wq = np.asarray(Wq, dtype=np.float32)
    Wk = np.asarray(Wk, dtype=np.float32)
    Wv = np.asarray(Wv, dtype=np.float32)
    Wo = np.asarray(Wo, dtype=np.float32)

    in_maps, has_bias = _in_maps(x, gamma, beta, Wq, Wk, Wv, Wo)
    nc = _get(has_bias)
    res = run_bass_kernel_spmd(nc, in_maps, core_ids=list(range(NCORES)))
    out = np.empty((B, N, D), dtype=np.float32)
    for c in range(NCORES):
        o = res.results[c]["out"]
        for k, g in enumerate(range(BPC * c, BPC * (c + 1))):
            out[g // 16, 128 * (g % 16):128 * (g % 16 + 1), :] = \
                o[128 * k:128 * (k + 1), :]
    return out



# revision 9
# speedup vs baseline: 1.2722x; 1.2722x over previous
"""Self-contained Trainium2 Bass kernel for nn_Attention_23776938951493.

Computation (see reference): LayerNorm -> q/k/v projections -> flat-reshape
attention (head h attends over tokens [128h, 128(h+1)) reshaped to [2048, 64])
-> out projection.  The flat reshape makes every (batch, head) pair an
independent 128-token block: 32 blocks total, 4 per NeuronCore, no
collectives needed.

v5: keeps the PE's HAM clock warm for the whole kernel (the v4 trace showed
a ~1.8us PE gap at each block boundary that re-throttled the clock to
1.2GHz for ~49us of every 90us block).  Structure:
  - all four blocks' LayerNorm activations hoisted to the start (2 ACT
    table loads total instead of 8)
  - q/k projections batched across the 4 blocks (N=512 moving instead of
    N=128), k-chunks pipelined so the first scores matmul fires ~13us in
  - scores accumulate in fp16 PSUM ([128,2,1024] per (hh,gp)) and a single
    2048-wide exp per (hh,gp) halves the ACT per-instruction overhead
  - softmax-normalize + out-projection of each half-block are deferred one
    attention window and woven between the next window's scores/AV matmuls
    so the PE queue never drains
"""

import os
import sys

sys.path.insert(0, "/opt/trn_rl_repo")
os.environ.setdefault("JAX_PLATFORMS", "axon")

import numpy as np
from contextlib import ExitStack

B, N, D = 2, 2048, 1024
H, DH = 16, 64
NCORES = 8
BLK = 128      # tokens per block
BPC = 4        # blocks per core
LN_EPS = 1e-5

_compiled = {}


def _build(has_bias: bool):
    import concourse.bass as bass
    import concourse.tile as tile
    from concourse import bacc, mybir
    from concourse.masks import make_identity

    f32 = mybir.dt.float32
    f32r = mybir.dt.float32r
    fp16 = mybir.dt.float16
    FT = mybir.ActivationFunctionType
    sub = mybir.AluOpType.subtract
    mult = mybir.AluOpType.mult

    nc = bacc.Bacc("TRN2", target_bir_lowering=False, debug=False,
                   num_devices=NCORES)
    xs = nc.dram_tensor("xs", [BPC * BLK, D], f32, kind="ExternalInput").ap()
    wdr = {}
    for nm in ("q", "k", "v", "o"):
        wdr[nm] = nc.dram_tensor(f"w{nm}", [D, D], fp16,
                                 kind="ExternalInput").ap()
    if has_bias:
        bqk_dr = nc.dram_tensor("bqk", [128, 16], f32,
                                kind="ExternalInput").ap()
        bv_dr = nc.dram_tensor("bv", [1, D], f32, kind="ExternalInput").ap()
    out_dr = nc.dram_tensor("out", [BPC * BLK, D], f32,
                            kind="ExternalOutput").ap()

    with tile.TileContext(nc) as tc:
        with ExitStack() as ctx:
            P = lambda name, bufs, **kw: ctx.enter_context(
                tc.tile_pool(name=name, bufs=bufs, **kw))
            consts = P("consts", 1)
            wpool = P("w", 1)
            xpool = P("x", 1)
            scrp = P("scr", 1)
            xnpool = P("xn", 1)
            xntp = P("xnt", 1)
            qkp = P("qk", 1)
            yvp = P("yv", 1)
            ep = P("e", 3)
            avsp = P("avs", 2)
            avp = P("av", 2)
            avtp = P("avt", 2)
            outp = P("out", 2)
            scsp = P("scs", 2, space="PSUM")   # 2 banks each
            flexp = P("flex", 1, space="PSUM")  # 2 banks
            psav = P("psav", 1, space="PSUM")   # 2 banks

            # identities for PE transposes
            idscratch = outp.tile([128, 1024], f32, tag="out")
            make_identity(nc, idscratch[:, 0:128])
            ident = consts.tile([128, 128], f32r, tag="ident")
            nc.vector.tensor_copy(out=ident[:], in_=idscratch[:, 0:128])
            ident16 = consts.tile([128, 128], fp16, tag="ident16")
            nc.vector.tensor_copy(out=ident16[:], in_=idscratch[:, 0:128])

            # resident fp16 weights, chunk-major; q first (needed ~9us in),
            # then k, v; o goes on the sync queue (not needed until ~50us)
            W = {}
            engs = [nc.gpsimd, nc.scalar]
            for nm in ("q", "k", "v", "o"):
                W[nm] = wpool.tile([128, 8, 1024], fp16, tag=f"w{nm}",
                                   name=f"w{nm}")
            for nm in ("q", "k", "v"):
                for j in range(8):
                    engs[j % 2].dma_start(
                        out=W[nm][:, j, :], in_=wdr[nm][128 * j:128 * (j + 1), :])
            for j in range(8):
                nc.sync.dma_start(
                    out=W["o"][:, j, :], in_=wdr["o"][128 * j:128 * (j + 1), :])
            if has_bias:
                bqk = consts.tile([128, 16], f32, tag="bqk")
                nc.sync.dma_start(out=bqk[:], in_=bqk_dr[:])
                bvb = consts.tile([128, D], f32, tag="bvb")
                nc.gpsimd.dma_start(out=bvb[:], in_=bv_dr.broadcast_to((128, D)))

            # ---- LayerNorm for all four blocks (hoisted: 2 table loads) ----
            xi = xpool.tile([128, BPC, D], f32, tag="x", name="xi")
            for b in range(BPC):
                nc.sync.dma_start(out=xi[:, b, :],
                                  in_=xs[BLK * b:BLK * (b + 1), :])
            scr = scrp.tile([128, BPC, 64], f32, tag="scr", name="scr")
            for b in range(BPC):
                nc.vector.memset(scr[:, b, 0:1], LN_EPS)
                sview = scr[:, b, 1:13].rearrange("p (s d) -> p s d", d=6)
                for s in range(2):
                    nc.vector.bn_stats(out=sview[:, s, :],
                                       in_=xi[:, b, 512 * s:512 * (s + 1)])
                nc.vector.bn_aggr(out=scr[:, b, 13:15], in_=sview)
            for b in range(BPC):   # ln(var+eps), grouped so the ACT table
                nc.scalar.activation(out=scr[:, b, 15:16],
                                     in_=scr[:, b, 14:15],
                                     func=FT.Ln, bias=scr[:, b, 0:1], scale=1.0)
            for b in range(BPC):   # rstd = exp(-0.5*ln(var+eps))
                nc.scalar.activation(out=scr[:, b, 30:31],
                                     in_=scr[:, b, 15:16],
                                     func=FT.Exp, scale=-0.5)
            xn = xnpool.tile([128, BPC, D], fp16, tag="xn", name="xn")
            for b in range(BPC):
                nc.vector.tensor_scalar(out=xn[:, b, :], in0=xi[:, b, :],
                                        scalar1=scr[:, b, 13:14],
                                        scalar2=scr[:, b, 30:31],
                                        op0=sub, op1=mult)

            # ---- xn transposed, j-chunk major: xnt[:, j, b, :] ----
            xnt = xntp.tile([128, 8, BPC, 128], fp16, tag="xnt", name="xnt")
            for j in range(8):
                for b in range(BPC):
                    # rotate pt through scs pool (idle here) + flex
                    pool = scsp if (j * BPC + b) % 3 != 2 else flexp
                    pt = pool.tile([128, 2048], fp16,
                                   tag="scs" if pool is scsp else "flexpt",
                                   name="pt")
                    nc.tensor.transpose(pt[:, 0:128],
                                        xn[:, b, 128 * j:128 * (j + 1)],
                                        ident16[:])
                    nc.vector.tensor_copy(out=xnt[:, j, b, :],
                                          in_=pt[:, 0:128])

            # ---- batched q/k projections (all 4 blocks, N=512) ----
            qt = qkp.tile([128, BPC, 16, 128], fp16, tag="qt", name="qt")
            kt = qkp.tile([128, BPC, 16, 128], fp16, tag="kt", name="kt")

            def qk_chunk(nm, ob):
                """channels 128*ob..128*ob+128 of projection nm, 4 blocks."""
                dst = qt if nm == "q" else kt
                pp = flexp.tile([128, 1024], f32, tag="flexpt", name="pp")
                for j in range(8):
                    nc.tensor.matmul(
                        pp[:, 0:512],
                        W[nm][:, j, 128 * ob:128 * (ob + 1)],
                        xnt[:, j, :, :], start=(j == 0), stop=(j == 7))
                if has_bias:
                    col = (0 if nm == "q" else 8) + ob
                    nc.vector.tensor_scalar_add(
                        out=pp[:, 0:512], in0=pp[:, 0:512],
                        scalar1=bqk[:, col:col + 1])
                ppv = pp[:, 0:512].rearrange("p (b t) -> p b t", b=BPC)
                # even head-chunk lands on partitions 0-63, odd on 64-127
                nc.vector.tensor_copy(out=dst[0:64, :, 2 * ob, :],
                                      in_=ppv[0:64, :, :])
                nc.vector.tensor_copy(out=dst[64:128, :, 2 * ob + 1, :],
                                      in_=ppv[64:128, :, :])
                nc.gpsimd.dma_start(out=dst[64:128, :, 2 * ob, :],
                                    in_=dst[0:64, :, 2 * ob, :])
                nc.sync.dma_start(out=dst[0:64, :, 2 * ob + 1, :],
                                  in_=dst[64:128, :, 2 * ob + 1, :])
                yield

            yva = yvp.tile([128, BPC, 16, 65], fp16, tag="yva", name="yva")

            def v_half(b, half):
                """v projection for block b, yva groups 8*half..8*half+8."""
                pv = flexp.tile([128, 1024], f32, tag="flexpt", name="pv")
                for j in range(8):
                    nc.tensor.matmul(
                        pv[:, 512 * half:512 * (half + 1)], xnt[:, j, b, :],
                        W["v"][:, j, 512 * half:512 * (half + 1)],
                        start=(j == 0), stop=(j == 7))
                yield
                if has_bias:
                    nc.vector.tensor_add(
                        out=pv[:, 512 * half:512 * (half + 1)],
                        in0=pv[:, 512 * half:512 * (half + 1)],
                        in1=bvb[:, 512 * half:512 * (half + 1)])
                if half == 0:
                    nc.vector.memset(yva[:, b, :, 64:65], 1.0)
                nc.vector.tensor_copy(
                    out=yva[:, b, 8 * half:8 * (half + 1), 0:64],
                    in_=pv[:, 512 * half:512 * (half + 1)].rearrange(
                        "p (g d) -> p g d", d=64))
                yield

            # per (b, hh) live state
            pa_t = {}
            avs_t = {}
            av_t = {}

            def attn_window(b, hh, gp, bg_pull):
                """scores+exp+AV for one (b,hh,gp); pulls bg work between."""
                pa = pa_t[(b, hh)]
                scs = [scsp.tile([128, 1024], f32, tag="scs", name=f"sc{s}")
                       for s in range(2)]
                e2 = ep.tile([128, 2, 8, 128], fp16, tag="e", name="e2")
                for q4 in range(2):
                    for side in range(2):
                        g2 = 2 * gp + side
                        base = 64 * side
                        nc.tensor.matmul(
                            scs[side][:, 512 * q4:512 * (q4 + 1)],
                            kt[base:base + 64, b, g2, :],
                            qt[base:base + 64, b,
                               8 * hh + 4 * q4:8 * hh + 4 * (q4 + 1), :],
                            start=True, stop=True)
                for side in range(2):
                    nc.scalar.activation(
                        out=e2[:, side, :, :].rearrange("p g q -> p (g q)"),
                        in_=scs[side][:], func=FT.Exp, scale=0.125)
                bg_pull()
                for side in range(2):
                    g2 = 2 * gp + side
                    for q4 in range(2):
                        nc.tensor.matmul(
                            pa[:, 4 * q4:4 * (q4 + 1), :],
                            yva[:, b, g2, :],
                            e2[:, side, 4 * q4:4 * (q4 + 1), :],
                            start=(gp == 0 and side == 0),
                            stop=(gp == 7 and side == 1))
                bg_pull()

            def avs_copy(b, hh):
                """drain pa -> SBUF right after (b,hh) attention (DVE only)."""
                avs = avsp.tile([128, 8, 128], f32r, tag="avs", name="avs")
                avs_t[(b, hh)] = avs
                nc.vector.tensor_copy(out=avs[0:65, :, :], in_=pa_t[(b, hh)][:])
                yield

            def norm_half(b, hh):
                """normalize avs(b,hh) into av(b) (8 transpose chunks)."""
                if hh == 0:
                    av = avp.tile([128, 1024], fp16, tag="av", name="av")
                    av_t[b] = av
                av = av_t[b]
                avs = avs_t[(b, hh)]
                for c in range(8):
                    tr = flexp.tile([128, 1024], f32, tag="flexpt", name="tr")
                    nc.tensor.transpose(tr[:, 0:128].bitcast(f32r),
                                        avs[:, c, :], ident[:])
                    rc = 32 + 8 * hh + c
                    nc.vector.reciprocal(out=scr[:, b, rc:rc + 1],
                                         in_=tr[:, 64:65])
                    g = 8 * hh + c
                    nc.vector.tensor_scalar_mul(
                        out=av[:, 64 * g:64 * (g + 1)],
                        in0=tr[:, 0:64], scalar1=scr[:, b, rc:rc + 1])
                    if c % 2:
                        yield

            def avt_out(b):
                """transpose av(b) and run the out projection."""
                av = av_t[b]
                avt = avtp.tile([128, 8, 128], fp16, tag="avt", name="avt")
                for j in range(8):
                    pt2 = flexp.tile([128, 2048], fp16, tag="flexpt",
                                     name="pt2")
                    nc.tensor.transpose(pt2[:, 0:128],
                                        av[:, 128 * j:128 * (j + 1)],
                                        ident16[:])
                    nc.vector.tensor_copy(out=avt[:, j, :], in_=pt2[:, 0:128])
                    if j % 2:
                        yield
                po = flexp.tile([128, 1024], f32, tag="flexpt", name="po")
                for hh in range(2):
                    for j in range(8):
                        nc.tensor.matmul(
                            po[:, 512 * hh:512 * (hh + 1)], avt[:, j, :],
                            W["o"][:, j, 512 * hh:512 * (hh + 1)],
                            start=(j == 0), stop=(j == 7))
                    yield
                ob_t = outp.tile([128, 1024], f32, tag="out", name="ob")
                nc.vector.tensor_copy(out=ob_t[:], in_=po[:])
                nc.sync.dma_start(out=out_dr[BLK * b:BLK * (b + 1), :],
                                  in_=ob_t[:])
                yield

            # ---- background work queue (strict emission order) ----
            bg = []

            def bg_pull(n=1):
                for _ in range(n):
                    while bg:
                        try:
                            next(bg[0])
                            return
                        except StopIteration:
                            bg.pop(0)

            def _mk_pull():
                return lambda: bg_pull(1)

            # pipelined startup: q-chunks for hh0 moving and block 0's v are
            # emitted eagerly; remaining k/q chunks are pulled between the
            # early windows (k(gp) always lands before window gp's scores)
            for ob in range(4):
                for _ in qk_chunk("q", ob):
                    pass
            for _ in qk_chunk("k", 0):
                pass
            for half in range(2):
                for _ in v_half(0, half):
                    pass
            bg.extend([qk_chunk("k", ob) for ob in range(1, 8)])
            bg.extend([qk_chunk("q", ob) for ob in range(4, 8)])

            pull = _mk_pull()
            for b in range(BPC):
                for hh in range(2):
                    # next block's v projection must be fully emitted before
                    # its first AV matmul -> queue it one half-block early
                    if hh == 1 and b + 1 < BPC:
                        bg.append(v_half(b + 1, 0))
                        bg.append(v_half(b + 1, 1))
                    pa_t[(b, hh)] = psav.tile([65, 8, 128], f32, tag="psav",
                                              name="pa")
                    for gp in range(8):
                        attn_window(b, hh, gp, pull)
                    # drain pa on DVE immediately; defer normalize one window
                    bg.insert(0, avs_copy(b, hh))
                    bg.append(norm_half(b, hh))
                    if hh == 1:
                        bg.append(avt_out(b))
            # tail
            while bg:
                try:
                    next(bg[0])
                except StopIteration:
                    bg.pop(0)

    nc.compile()
    return nc


def _get(has_bias: bool):
    if has_bias not in _compiled:
        _compiled[has_bias] = _build(has_bias)
    return _compiled[has_bias]


def _in_maps(x, gamma, beta, Wq, Wk, Wv, Wo):
    wq_t = np.ascontiguousarray((Wq * gamma[None, :]).T.astype(np.float16))
    wk_t = np.ascontiguousarray((Wk * gamma[None, :]).T.astype(np.float16))
    wv_t = np.ascontiguousarray((Wv * gamma[None, :]).T.astype(np.float16))
    wo_t = np.ascontiguousarray(Wo.T.astype(np.float16))
    has_bias = bool(np.any(beta))
    maps = []
    for c in range(NCORES):
        blocks = [x[g // 16, 128 * (g % 16):128 * (g % 16 + 1), :]
                  for g in range(BPC * c, BPC * (c + 1))]
        m = {"xs": np.ascontiguousarray(np.concatenate(blocks, axis=0)),
             "wq": wq_t, "wk": wk_t, "wv": wv_t, "wo": wo_t}
        if has_bias:
            bq = beta @ Wq.T
            bk = beta @ Wk.T
            bv = beta @ Wv.T
            m["bqk"] = np.ascontiguousarray(np.concatenate(
                [bq.reshape(8, 128).T, bk.reshape(8, 128).T], axis=1))
            m["bv"] = np.ascontiguousarray(bv.reshape(1, D))
        maps.append(m)
    return maps, has_bias


def kernel(x, gamma, beta, Wq, Wk, Wv, Wo):
    from concourse.bass_utils import run_bass_kernel_spmd

    x = np.ascontiguousarray(np.asarray(x, dtype=np.float32))
    gamma = np.asarray(gamma, dtype=np.float32)
    beta = np.asarray(beta, dtype=np.float32)
    Wq = np.asarray(Wq, dtype=np.float32)
    Wk = np.asarray(Wk, dtype=np.float32)
    Wv = np.asarray(Wv, dtype=np.float32)
    Wo = np.asarray(Wo, dtype=np.float32)

    in_maps, has_bias = _in_maps(x, gamma, beta, Wq, Wk, Wv, Wo)
    nc = _get(has_bias)
    res = run_bass_kernel_spmd(nc, in_maps, core_ids=list(range(NCORES)))
    out = np.empty((B, N, D), dtype=np.float32)
    for c in range(NCORES):
        o = res.results[c]["out"]
        for k, g in enumerate(range(BPC * c, BPC * (c + 1))):
            out[g // 16, 128 * (g % 16):128 * (g % 16 + 1), :] = \
                o[128 * k:128 * (k + 1), :]
    return out
